# revision 45
# baseline (speedup 1.0000x reference)
"""Trainium2 Bass kernel for motion-resolved NUFFT-adjoint (gpuNUFFT adj + warp).

Sharding: 8 cores = 4 motion states x 2 m-halves (each core: all 8 coils,
M/2=4096 k-space samples of one frame). Each core computes its partial
SENSE image and applies the frame's warp-adjoint (bilinear scatter-add,
expressed as two hat-matrix matmuls on the TensorEngine). Host sums the 8
partial outputs — no collectives needed since everything is linear in the
k-space data.

Device pipeline per core:
  A. phases psi = kx*(x-64) + 64 per m-chunk (DVE); range reduction
     t = psi - rint(psi) via the 2^23 magic-add (Pool, split into two
     single-op instructions: the chained-op ALU keeps an unrounded
     intermediate, and only positive sums round IEEE-correctly);
     sin(2*pi*t) on ACT Sin (valid input range [-pi, pi]), bf16 out
  B. fold: raw products P1..P4 = {eyr,eyi} x {dr,di}_c[m] only, via
     DVE tensor_scalar_mul / ACT Copy-with-scale-AP (bf16) - no
     elementwise complex combine at all
  C. NUFFT: the complex combine runs as 8 fp32 PSUM-accumulation
     chains on the PE with weight set {exr, exi, -exr, -exi} (the
     negated copies are free extra ACT sins with scale=-2pi):
       im_r += exr^T P1 - exr^T P2 - exi^T P3 - exi^T P4
       im_i += exr^T P3 + exr^T P4 + exi^T P1 - exi^T P2
  D. SENSE: img = sum_c conj(csm_c)*im_c, wide mults + coil-tree (DVE)
  E. warp adjoint: neg-hat hx = min(|pxf-px|,1)-1 (DVE+Pool) and
     pos-hat hy = relu(1-|pyf-py|) (ACT); R = hy * (-img_col);
     out += hx^T @ R accumulates over 128 q-chunks (PE), so the
     bilinear scatter-add runs entirely on the TensorEngine.

Engine-level notes baked in: every TPB instruction gets exactly one
sync-wait slot in this walrus (extra waits hoisted onto NoOps by
_split_multiwaits); Pool/ACT reject AP-scalar tensor_scalar, so all
per-partition-scalar elementwise work lives on DVE with immediate-only
passes on Pool and affine/bias-AP passes on ACT; hat emission is placed
after the fold so the Tile scheduler does not starve the NUFFT chain.
"""
import sys
import numpy as np

sys.path.insert(0, "/opt/trn_rl_repo")

import concourse.bass as bass
import concourse.mybir as mybir
import concourse.tile as tile
from concourse.bass_utils import run_bass_kernel_spmd

F32 = mybir.dt.float32
BF16 = mybir.dt.bfloat16
Alu = mybir.AluOpType
Act = mybir.ActivationFunctionType

Nc, M, Nt, Nx, Ny = 8, 8192, 4, 128, 128
MH = M // 2          # m-samples per core
NCH = MH // 128      # 32 m-chunks per core
NQ = Nx * Ny // 128  # 128 q-chunks (warp contraction)
TWO_PI = float(2.0 * np.pi)
PI = float(np.pi)
CLIP_HI = float(np.float32(Nx - 1.001))

SEGS = [("kx", NCH), ("ky", NCH), ("dr", Nc * NCH), ("di", Nc * NCH),
        ("csr", Nc * 128), ("csi", Nc * 128),
        ("fx", 128), ("fy", 128), ("ramp", 128), ("rampc", 128),
        ("xcol", 1), ("bnpi", 1), ("bppi", 1)]
OFFS = {}
_o = 0
for _n, _w in SEGS:
    OFFS[_n] = _o
    _o += _w
NCOLS = _o

_PROGRAM = None


def _split_multiwaits(nc):
    """Walrus codegen fits one sync-wait per TPB instruction; hoist extras
    into single-wait NoOps on the same engine, placed just before."""
    for bb in nc.main_func.blocks:
        new = []
        for ins in bb.instructions:
            si = getattr(ins, "sync_info", None)
            if si is not None and si.on_wait and len(si.on_wait) > 1:
                waits = list(si.on_wait)
                for w in waits[:-1]:
                    nop = mybir.InstNoOp(name=f"WSPLIT-{nc.next_id()}", ins=[], outs=[])
                    nop.engine = ins.engine
                    nop.sync_info = mybir.SyncInfo(on_wait=[w], on_update=[])
                    new.append(nop)
                ins.sync_info = mybir.SyncInfo(
                    on_wait=[waits[-1]], on_update=list(si.on_update))
            new.append(ins)
        bb.instructions[:] = new


def _build_program():
    nc = bass.Bass()

    inp = nc.declare_dram_parameter("inp", [128, NCOLS], F32, isOutput=False)
    out = nc.declare_dram_parameter("out", [128, 256], F32, isOutput=True)

    with tile.TileContext(nc) as tc:
        with (
            tc.tile_pool(name="const", bufs=1) as cpool,
            tc.tile_pool(name="big", bufs=1) as bpool,
            tc.tile_pool(name="ph", bufs=1) as phpool,
            tc.tile_pool(name="fold", bufs=4) as fpool,
            tc.tile_pool(name="warp", bufs=7) as wpool,
            tc.tile_pool(name="psum", bufs=1, space="PSUM") as pspool,
        ):
            # ---- single input DMA; all operands are slices of inp_sb ----
            inp_sb = cpool.tile([128, NCOLS], F32, tag="inp")
            nc.sync.dma_start(inp_sb[:], inp[:])
            # downstream ops otherwise each wait on several DMA-queue sems and
            # overflow the per-instruction sync-wait budget in walrus codegen
            tc.strict_bb_all_engine_barrier()

            def seg(name, n):
                o = OFFS[name]
                return inp_sb[:, o:o + n]

            kx_sb = seg("kx", NCH)
            ky_sb = seg("ky", NCH)
            dr_sb = seg("dr", Nc * NCH)
            di_sb = seg("di", Nc * NCH)
            csr_sb = seg("csr", Nc * 128)
            csi_sb = seg("csi", Nc * 128)
            fx_sb = seg("fx", 128)
            fy_sb = seg("fy", 128)
            ramp = seg("ramp", 128)     # row j -> j
            rampc = seg("rampc", 128)   # row j -> j-64
            xcol = seg("xcol", 1)       # partition p -> p

            # ---- stage A: phase tables + sincos ----
            MAGIC = float(2.0 ** 23)

            def build_exp(ktraj_sb, want_neg, inter):
                """bf16 sin/cos tables [128, MH] (m=128k+p, col k*128+x).
                psi = kx*(x-64) + 64 kept positive: the Pool/DVE adders only
                match IEEE round-to-nearest for positive magic-number sums."""
                ph = phpool.tile([128, MH], F32, tag="ph")
                for k in range(NCH):
                    nc.vector.tensor_scalar(
                        ph[:, k * 128:(k + 1) * 128], rampc,
                        ktraj_sb[:, k:k + 1], 64.0, Alu.mult, Alu.add)
                if inter is None:
                    ei = bpool.tile([128, MH], BF16, tag=f"ei{want_neg}")
                    er = bpool.tile([128, MH], BF16, tag=f"er{want_neg}")
                else:
                    # strided views into the interleaved tile: er -> even
                    # 128-col blocks, ei -> odd blocks (multi-dim APs)
                    v = inter[:].rearrange("p (k two x) -> p k two x", two=2, x=128)
                    er = v[:, :, 0, :]
                    ei = v[:, :, 1, :]
                if want_neg:
                    nei = bpool.tile([128, MH], BF16, tag="nei")
                    ner = bpool.tile([128, MH], BF16, tag="ner")
                else:
                    nei = None
                    ner = None
                QW = MH // 4
                for q in range(4):  # quarters pipeline Pool->DVE->ACT stages
                    qs = slice(q * QW, (q + 1) * QW)
                    phc = phpool.tile([128, QW], F32, tag="phcq")
                    nc.gpsimd.tensor_scalar_add(phc[:], ph[:, qs], 0.25)
                    t_s = phpool.tile([128, QW], F32, tag="tsq")
                    t_c = phpool.tile([128, QW], F32, tag="tcq")
                    for psi, t in ((ph[:, qs], t_s[:]), (phc[:], t_c[:])):
                        # k = rint(psi) via the 2^23 trick; two separate
                        # instructions so the write rounds to fp32
                        kk = phpool.tile([128, QW], F32, tag="kk")
                        nc.gpsimd.tensor_scalar_add(kk[:], psi, MAGIC)
                        nc.gpsimd.tensor_scalar_sub(kk[:], kk[:], MAGIC)
                        nc.gpsimd.tensor_sub(t, psi, kk[:])
                    if inter is None:
                        ei_q, er_q = ei[:, qs], er[:, qs]
                    else:
                        ei_q, er_q = ei[:, q * 8:(q + 1) * 8, :], er[:, q * 8:(q + 1) * 8, :]
                    nc.scalar.activation(ei_q, t_s[:], Act.Sin, bias=0.0, scale=TWO_PI)
                    nc.scalar.activation(er_q, t_c[:], Act.Sin, bias=0.0, scale=TWO_PI)
                    if want_neg:
                        nc.scalar.activation(nei[:, qs], t_s[:], Act.Sin, bias=0.0, scale=-TWO_PI)
                        nc.scalar.activation(ner[:, qs], t_c[:], Act.Sin, bias=0.0, scale=-TWO_PI)
                return er, ei, nei, ner

            # ey first: the fold (DVE-heavy) only needs ey; ex (matmul lhsT)
            # builds concurrently with the early fold chunks.
            # ey lands interleaved per chunk ([eyr_k | eyi_k] 256-col blocks)
            # so one FD=256 tensor_scalar yields two fold products at once.
            ey2 = bpool.tile([128, NCH * 256], BF16, tag="ey2")
            build_exp(ky_sb, False, ey2)
            exr, exi, exnegi, exnegr = build_exp(kx_sb, True, None)

            # ---- stages B+C: fold + NUFFT matmuls ----
            # raw products P1..P4; the complex combines run as 8 PSUM
            # accumulation chains on the PE (weights exr/exi/-exr/-exi)
            # im_r = exr*P1 - exr*P2 - exi*P3 - exi*P4
            # im_i = exr*P3 + exr*P4 + exi*P1 - exi*P2
            ps_r = pspool.tile([128, Nc * 128], F32, tag="ps_r")
            ps_i = pspool.tile([128, Nc * 128], F32, tag="ps_i")
            for k in range(NCH):
                bsl = slice(k * 256, (k + 1) * 256)
                P13 = fpool.tile([128, Nc * 256], BF16, tag="P13")
                P42 = fpool.tile([128, Nc * 256], BF16, tag="P42")
                for c in range(Nc):
                    csl = slice(c * 256, (c + 1) * 256)
                    dcol = dr_sb[:, c * NCH + k:c * NCH + k + 1]
                    icol = di_sb[:, c * NCH + k:c * NCH + k + 1]
                    if c < 2 and not (c == 0 and k % 3 == 0):
                        nc.scalar.activation(P13[:, csl], ey2[:, bsl], Act.Copy, scale=dcol)
                        nc.scalar.activation(P42[:, csl], ey2[:, bsl], Act.Copy, scale=icol)
                    else:
                        nc.vector.tensor_scalar_mul(P13[:, csl], ey2[:, bsl], dcol)
                        nc.vector.tensor_scalar_mul(P42[:, csl], ey2[:, bsl], icol)

                def half(buf, off, h):
                    # coils 4h..4h+3, inner 128 cols at `off` within each
                    # coil's 256-block -> [128, 4, 128] strided rhs (free 512)
                    v = buf[:].rearrange("p (c two x) -> p c two x", two=2, x=128)
                    return v[:, 4 * h:4 * h + 4, off, :]

                first, last = (k == 0), (k == NCH - 1)
                for h in range(2):  # N=512 column halves (4 coils each)
                    P1h, P3h = half(P13, 0, h), half(P13, 1, h)
                    P4h, P2h = half(P42, 0, h), half(P42, 1, h)
                    hsl = slice(h * 512, (h + 1) * 512)
                    msl = slice(k * 128, (k + 1) * 128)
                    nc.tensor.matmul(ps_r[:, hsl], exr[:, msl], P1h,
                                     start=first, stop=False)
                    nc.tensor.matmul(ps_i[:, hsl], exr[:, msl], P3h,
                                     start=first, stop=False)
                    nc.tensor.matmul(ps_i[:, hsl], exr[:, msl], P4h,
                                     start=False, stop=False)
                    nc.tensor.matmul(ps_r[:, hsl], exnegr[:, msl], P2h,
                                     start=False, stop=False)
                    nc.tensor.matmul(ps_r[:, hsl], exnegi[:, msl], P3h,
                                     start=False, stop=False)
                    nc.tensor.matmul(ps_r[:, hsl], exnegi[:, msl], P4h,
                                     start=False, stop=last)
                    nc.tensor.matmul(ps_i[:, hsl], exi[:, msl], P1h,
                                     start=False, stop=False)
                    nc.tensor.matmul(ps_i[:, hsl], exnegi[:, msl], P2h,
                                     start=False, stop=last)

            # ---- warp coords + hat matrices (overlap NUFFT on PE) ----
            pxf = cpool.tile([128, 128], F32, tag="pxf")
            nc.vector.tensor_scalar(pxf[:], fx_sb, xcol, 0.0, Alu.add, Alu.max)
            nc.vector.tensor_scalar_min(pxf[:], pxf[:], CLIP_HI)
            pyf = cpool.tile([128, 128], F32, tag="pyf")
            nc.vector.tensor_add(pyf[:], fy_sb, ramp)
            nc.vector.tensor_scalar(pyf[:], pyf[:], 0.0, CLIP_HI, Alu.max, Alu.min)
            pyf_neg = cpool.tile([128, 128], F32, tag="pyfn")
            nc.gpsimd.tensor_scalar_mul(pyf_neg[:], pyf[:], -1.0)

            # hx = min(|pxf - px|, 1) - 1 (NEGATED hat, DVE+Pool)
            # hy = relu(1 - |pyf - py|)  (positive hat, ACT)
            # sign balance: R is scaled by -img, so (-hx)^T @ (hy * -img) = +.
            hx_all = bpool.tile([128, NQ * 128], BF16, tag="hx")
            hy_all = bpool.tile([128, NQ * 128], BF16, tag="hy")
            for k in range(NQ):
                sl = slice(k * 128, (k + 1) * 128)
                vx = wpool.tile([128, 128], F32, tag="vx")
                nc.vector.tensor_scalar_sub(vx[:], ramp, pxf[:, k:k + 1])
                nc.vector.scalar_tensor_tensor(   # |vx| in place
                    vx[:], vx[:], -1.0, vx[:], Alu.mult, Alu.max)
                nc.gpsimd.tensor_scalar(
                    hx_all[:, sl], vx[:], 1.0, 1.0, Alu.min, Alu.subtract)
                h1y = wpool.tile([128, 128], F32, tag="h1y")
                nc.scalar.activation(
                    h1y[:], ramp, Act.Abs, bias=pyf_neg[:, k:k + 1], scale=1.0)
                nc.scalar.activation(
                    hy_all[:, sl], h1y[:], Act.Relu, bias=1.0, scale=-1.0)

            # ---- stage D: SENSE coil combine (wide mult + coil-tree add) ----
            def coil_combine(ps_a, cs_a, ps_b, cs_b, op_b, out_t):
                """out = sum_c cs_a*ps_a (+/-) cs_b*ps_b, tree-reduced over 8 coils."""
                w = cpool.tile([128, Nc * 128], F32, tag="sensew")
                nc.vector.tensor_tensor(w[:], ps_a[:], cs_a, Alu.mult)
                w2 = pspool.tile([128, Nc * 128], F32, tag="sensew2")
                nc.vector.tensor_tensor(w2[:], ps_b[:], cs_b, Alu.mult)
                nc.vector.tensor_tensor(w[:], w[:], w2[:], op_b)
                nc.vector.tensor_tensor(w[:, 0:512], w[:, 0:512], w[:, 512:1024], Alu.add)
                nc.vector.tensor_tensor(w[:, 0:256], w[:, 0:256], w[:, 256:512], Alu.add)
                nc.vector.tensor_tensor(out_t[:], w[:, 0:128], w[:, 128:256], Alu.add)

            img_r = cpool.tile([128, 128], F32, tag="imgr")
            img_i = cpool.tile([128, 128], F32, tag="imgi")
            coil_combine(ps_r, csr_sb, ps_i, csi_sb, Alu.add, img_r)
            coil_combine(ps_i, csr_sb, ps_r, csi_sb, Alu.subtract, img_i)

            # ---- stage E: warp-adjoint matmuls ----
            imgr_n = cpool.tile([128, 128], F32, tag="imgrn")
            nc.gpsimd.tensor_scalar_mul(imgr_n[:], img_r[:], -1.0)
            imgi_n = cpool.tile([128, 128], F32, tag="imgin")
            nc.gpsimd.tensor_scalar_mul(imgi_n[:], img_i[:], -1.0)
            ps_out = pspool.tile([128, 256], F32, tag="ps_out")
            for k in range(NQ):
                sl = slice(k * 128, (k + 1) * 128)
                R = wpool.tile([128, 256], BF16, tag="R")
                nc.vector.tensor_scalar_mul(R[:, 0:128], hy_all[:, sl], imgr_n[:, k:k + 1])
                nc.vector.tensor_scalar_mul(R[:, 128:256], hy_all[:, sl], imgi_n[:, k:k + 1])
                nc.tensor.matmul(ps_out[:], hx_all[:, sl], R[:],
                                 start=(k == 0), stop=(k == NQ - 1))

            out_sb = cpool.tile([128, 256], F32, tag="out_sb")
            nc.scalar.copy(out_sb[:], ps_out[:])
            nc.sync.dma_start(out[:], out_sb[:])

    _split_multiwaits(nc)
    return nc


def _prep_inputs(kspace, traj, csm, dcf, flow):
    """Shard full inputs into 8 per-core input maps (host-side, mechanical)."""
    kspace = np.asarray(kspace)
    traj = np.asarray(traj, dtype=np.float32)
    csm = np.asarray(csm)
    dcf = np.asarray(dcf, dtype=np.float32)
    flow = np.asarray(flow, dtype=np.float32)

    csr = np.ascontiguousarray(
        csm[0].real.astype(np.float32).transpose(1, 0, 2).reshape(128, Nc * 128))
    csi = np.ascontiguousarray(
        csm[0].imag.astype(np.float32).transpose(1, 0, 2).reshape(128, Nc * 128))

    f32 = np.float32
    ramp = np.broadcast_to(np.arange(128, dtype=f32)[None, :], (128, 128))
    rampc = ramp - f32(64.0)
    xcol = np.arange(128, dtype=f32)[:, None]
    bnpi = np.full((128, 1), -np.pi, f32)
    bppi = np.full((128, 1), np.pi, f32)

    in_maps = []
    for core in range(8):
        t, h = core // 2, core % 2
        msl = slice(h * MH, (h + 1) * MH)
        kx = traj[0, msl, 0, t].reshape(NCH, 128).T
        ky = traj[0, msl, 1, t].reshape(NCH, 128).T
        d = kspace[0, :, msl] * dcf[0, msl, t][None, :]
        dr = d.real.astype(f32).reshape(Nc, NCH, 128).transpose(2, 0, 1).reshape(128, Nc * NCH)
        di = d.imag.astype(f32).reshape(Nc, NCH, 128).transpose(2, 0, 1).reshape(128, Nc * NCH)
        inp = np.concatenate(
            [kx, ky, dr, di, csr, csi,
             flow[0, :, :, 0, t], flow[0, :, :, 1, t],
             ramp, rampc, xcol, bnpi, bppi], axis=1, dtype=f32)
        assert inp.shape == (128, NCOLS)
        in_maps.append({"inp": np.ascontiguousarray(inp)})
    return in_maps


def _run(inputs, trace=False):
    global _PROGRAM
    if _PROGRAM is None:
        _PROGRAM = _build_program()
    in_maps = _prep_inputs(**inputs)
    res = run_bass_kernel_spmd(_PROGRAM, in_maps, list(range(8)), trace=trace)
    acc = np.zeros((128, 256), np.float64)
    for r in res.results:
        acc += r["out"].astype(np.float64)
    acc = acc.astype(np.float32)
    full = np.stack([acc[:, 0:128], acc[:, 128:256]], axis=-1)[None]
    return full, res


def kernel(kspace, traj, csm, dcf, flow):
    full, _ = _run(dict(kspace=kspace, traj=traj, csm=csm, dcf=dcf, flow=flow))
    return full


# revision 46
# speedup vs baseline: 1.0021x; 1.0021x over previous
"""Trainium2 Bass kernel for motion-resolved NUFFT-adjoint (gpuNUFFT adj + warp).

Sharding: 8 cores = 4 motion states x 2 m-halves (each core: all 8 coils,
M/2=4096 k-space samples of one frame). Each core computes its partial
SENSE image and applies the frame's warp-adjoint (bilinear scatter-add,
expressed as two hat-matrix matmuls on the TensorEngine). Host sums the 8
partial outputs — no collectives needed since everything is linear in the
k-space data.

Device pipeline per core:
  A. phases psi = kx*(x-64) + 64 per m-chunk (DVE); range reduction
     t = psi - rint(psi) via the 2^23 magic-add (Pool, split into two
     single-op instructions: the chained-op ALU keeps an unrounded
     intermediate, and only positive sums round IEEE-correctly);
     sin(2*pi*t) on ACT Sin (valid input range [-pi, pi]), bf16 out
  B. fold: raw products P1..P4 = {eyr,eyi} x {dr,di}_c[m] only, via
     DVE tensor_scalar_mul / ACT Copy-with-scale-AP (bf16) - no
     elementwise complex combine at all
  C. NUFFT: the complex combine runs as 8 fp32 PSUM-accumulation
     chains on the PE with weight set {exr, exi, -exr, -exi} (the
     negated copies are free extra ACT sins with scale=-2pi):
       im_r += exr^T P1 - exr^T P2 - exi^T P3 - exi^T P4
       im_i += exr^T P3 + exr^T P4 + exi^T P1 - exi^T P2
  D. SENSE: img = sum_c conj(csm_c)*im_c, wide mults + coil-tree (DVE)
  E. warp adjoint: neg-hat hx = min(|pxf-px|,1)-1 (DVE+Pool) and
     pos-hat hy = relu(1-|pyf-py|) (ACT); R = hy * (-img_col);
     out += hx^T @ R accumulates over 128 q-chunks (PE), so the
     bilinear scatter-add runs entirely on the TensorEngine.

Engine-level notes baked in: every TPB instruction gets exactly one
sync-wait slot in this walrus (extra waits hoisted onto NoOps by
_split_multiwaits); Pool/ACT reject AP-scalar tensor_scalar, so all
per-partition-scalar elementwise work lives on DVE with immediate-only
passes on Pool and affine/bias-AP passes on ACT; hat emission is placed
after the fold so the Tile scheduler does not starve the NUFFT chain.
"""
import sys
import numpy as np

sys.path.insert(0, "/opt/trn_rl_repo")

import concourse.bass as bass
import concourse.mybir as mybir
import concourse.tile as tile
from concourse.bass_utils import run_bass_kernel_spmd

F32 = mybir.dt.float32
BF16 = mybir.dt.bfloat16
Alu = mybir.AluOpType
Act = mybir.ActivationFunctionType

Nc, M, Nt, Nx, Ny = 8, 8192, 4, 128, 128
MH = M // 2          # m-samples per core
NCH = MH // 128      # 32 m-chunks per core
NQ = Nx * Ny // 128  # 128 q-chunks (warp contraction)
TWO_PI = float(2.0 * np.pi)
PI = float(np.pi)
CLIP_HI = float(np.float32(Nx - 1.001))

SEGS = [("kx", NCH), ("ky", NCH), ("dr", Nc * NCH), ("di", Nc * NCH),
        ("csr", Nc * 128), ("csi", Nc * 128),
        ("fx", 128), ("fy", 128), ("ramp", 128), ("rampc", 128),
        ("xcol", 1), ("bnpi", 1), ("bppi", 1)]
OFFS = {}
_o = 0
for _n, _w in SEGS:
    OFFS[_n] = _o
    _o += _w
NCOLS = _o

_PROGRAM = None


def _split_multiwaits(nc):
    """Walrus codegen fits one sync-wait per TPB instruction; hoist extras
    into single-wait NoOps on the same engine, placed just before."""
    for bb in nc.main_func.blocks:
        new = []
        for ins in bb.instructions:
            si = getattr(ins, "sync_info", None)
            if si is not None and si.on_wait and len(si.on_wait) > 1:
                waits = list(si.on_wait)
                for w in waits[:-1]:
                    nop = mybir.InstNoOp(name=f"WSPLIT-{nc.next_id()}", ins=[], outs=[])
                    nop.engine = ins.engine
                    nop.sync_info = mybir.SyncInfo(on_wait=[w], on_update=[])
                    new.append(nop)
                ins.sync_info = mybir.SyncInfo(
                    on_wait=[waits[-1]], on_update=list(si.on_update))
            new.append(ins)
        bb.instructions[:] = new


def _build_program():
    nc = bass.Bass()

    inp = nc.declare_dram_parameter("inp", [128, NCOLS], F32, isOutput=False)
    out = nc.declare_dram_parameter("out", [128, 256], F32, isOutput=True)

    with tile.TileContext(nc) as tc:
        with (
            tc.tile_pool(name="const", bufs=1) as cpool,
            tc.tile_pool(name="big", bufs=1) as bpool,
            tc.tile_pool(name="ph", bufs=1) as phpool,
            tc.tile_pool(name="fold", bufs=3) as fpool,
            tc.tile_pool(name="warp", bufs=10) as wpool,
            tc.tile_pool(name="psum", bufs=1, space="PSUM") as pspool,
        ):
            # ---- single input DMA; all operands are slices of inp_sb ----
            inp_sb = cpool.tile([128, NCOLS], F32, tag="inp")
            nc.sync.dma_start(inp_sb[:], inp[:])
            # downstream ops otherwise each wait on several DMA-queue sems and
            # overflow the per-instruction sync-wait budget in walrus codegen
            tc.strict_bb_all_engine_barrier()

            def seg(name, n):
                o = OFFS[name]
                return inp_sb[:, o:o + n]

            kx_sb = seg("kx", NCH)
            ky_sb = seg("ky", NCH)
            dr_sb = seg("dr", Nc * NCH)
            di_sb = seg("di", Nc * NCH)
            csr_sb = seg("csr", Nc * 128)
            csi_sb = seg("csi", Nc * 128)
            fx_sb = seg("fx", 128)
            fy_sb = seg("fy", 128)
            ramp = seg("ramp", 128)     # row j -> j
            rampc = seg("rampc", 128)   # row j -> j-64
            xcol = seg("xcol", 1)       # partition p -> p

            # ---- stage A: phase tables + sincos ----
            MAGIC = float(2.0 ** 23)

            def build_exp(ktraj_sb, want_neg, inter):
                """bf16 sin/cos tables [128, MH] (m=128k+p, col k*128+x).
                psi = kx*(x-64) + 64 kept positive: the Pool/DVE adders only
                match IEEE round-to-nearest for positive magic-number sums."""
                ph = phpool.tile([128, MH], F32, tag="ph")
                for k in range(NCH):
                    nc.vector.tensor_scalar(
                        ph[:, k * 128:(k + 1) * 128], rampc,
                        ktraj_sb[:, k:k + 1], 64.0, Alu.mult, Alu.add)
                if inter is None:
                    ei = bpool.tile([128, MH], BF16, tag=f"ei{want_neg}")
                    er = bpool.tile([128, MH], BF16, tag=f"er{want_neg}")
                else:
                    # strided views into the interleaved tile: er -> even
                    # 128-col blocks, ei -> odd blocks (multi-dim APs)
                    v = inter[:].rearrange("p (k two x) -> p k two x", two=2, x=128)
                    er = v[:, :, 0, :]
                    ei = v[:, :, 1, :]
                if want_neg:
                    nei = bpool.tile([128, MH], BF16, tag="nei")
                    ner = bpool.tile([128, MH], BF16, tag="ner")
                else:
                    nei = None
                    ner = None
                QW = MH // 4
                for q in range(4):  # quarters pipeline Pool->DVE->ACT stages
                    qs = slice(q * QW, (q + 1) * QW)
                    phc = phpool.tile([128, QW], F32, tag="phcq")
                    nc.gpsimd.tensor_scalar_add(phc[:], ph[:, qs], 0.25)
                    t_s = phpool.tile([128, QW], F32, tag="tsq")
                    t_c = phpool.tile([128, QW], F32, tag="tcq")
                    for psi, t in ((ph[:, qs], t_s[:]), (phc[:], t_c[:])):
                        # k = rint(psi) via the 2^23 trick; two separate
                        # instructions so the write rounds to fp32
                        kk = phpool.tile([128, QW], F32, tag="kk")
                        nc.gpsimd.tensor_scalar_add(kk[:], psi, MAGIC)
                        nc.gpsimd.tensor_scalar_sub(kk[:], kk[:], MAGIC)
                        nc.gpsimd.tensor_sub(t, psi, kk[:])
                    if inter is None:
                        ei_q, er_q = ei[:, qs], er[:, qs]
                    else:
                        ei_q, er_q = ei[:, q * 8:(q + 1) * 8, :], er[:, q * 8:(q + 1) * 8, :]
                    nc.scalar.activation(ei_q, t_s[:], Act.Sin, bias=0.0, scale=TWO_PI)
                    nc.scalar.activation(er_q, t_c[:], Act.Sin, bias=0.0, scale=TWO_PI)
                    if want_neg:
                        nc.scalar.activation(nei[:, qs], t_s[:], Act.Sin, bias=0.0, scale=-TWO_PI)
                        nc.scalar.activation(ner[:, qs], t_c[:], Act.Sin, bias=0.0, scale=-TWO_PI)
                return er, ei, nei, ner

            # ey first: the fold (DVE-heavy) only needs ey; ex (matmul lhsT)
            # builds concurrently with the early fold chunks.
            # ey lands interleaved per chunk ([eyr_k | eyi_k] 256-col blocks)
            # so one FD=256 tensor_scalar yields two fold products at once.
            ey2 = bpool.tile([128, NCH * 256], BF16, tag="ey2")
            build_exp(ky_sb, False, ey2)
            exr, exi, exnegi, exnegr = build_exp(kx_sb, True, None)

            # ---- stages B+C: fold + NUFFT matmuls ----
            # raw products P1..P4; the complex combines run as 8 PSUM
            # accumulation chains on the PE (weights exr/exi/-exr/-exi)
            # im_r = exr*P1 - exr*P2 - exi*P3 - exi*P4
            # im_i = exr*P3 + exr*P4 + exi*P1 - exi*P2
            ps_r = pspool.tile([128, Nc * 128], F32, tag="ps_r")
            ps_i = pspool.tile([128, Nc * 128], F32, tag="ps_i")
            for k in range(NCH):
                bsl = slice(k * 256, (k + 1) * 256)
                P13 = fpool.tile([128, Nc * 256], BF16, tag="P13")
                P42 = fpool.tile([128, Nc * 256], BF16, tag="P42")
                for c in range(Nc):
                    csl = slice(c * 256, (c + 1) * 256)
                    dcol = dr_sb[:, c * NCH + k:c * NCH + k + 1]
                    icol = di_sb[:, c * NCH + k:c * NCH + k + 1]
                    if c < 2 and not (c == 0 and k % 3 == 0):
                        nc.scalar.activation(P13[:, csl], ey2[:, bsl], Act.Copy, scale=dcol)
                        nc.scalar.activation(P42[:, csl], ey2[:, bsl], Act.Copy, scale=icol)
                    else:
                        nc.vector.tensor_scalar_mul(P13[:, csl], ey2[:, bsl], dcol)
                        nc.vector.tensor_scalar_mul(P42[:, csl], ey2[:, bsl], icol)

                def half(buf, off, h):
                    # coils 4h..4h+3, inner 128 cols at `off` within each
                    # coil's 256-block -> [128, 4, 128] strided rhs (free 512)
                    v = buf[:].rearrange("p (c two x) -> p c two x", two=2, x=128)
                    return v[:, 4 * h:4 * h + 4, off, :]

                first, last = (k == 0), (k == NCH - 1)
                for h in range(2):  # N=512 column halves (4 coils each)
                    P1h, P3h = half(P13, 0, h), half(P13, 1, h)
                    P4h, P2h = half(P42, 0, h), half(P42, 1, h)
                    hsl = slice(h * 512, (h + 1) * 512)
                    msl = slice(k * 128, (k + 1) * 128)
                    nc.tensor.matmul(ps_r[:, hsl], exr[:, msl], P1h,
                                     start=first, stop=False)
                    nc.tensor.matmul(ps_i[:, hsl], exr[:, msl], P3h,
                                     start=first, stop=False)
                    nc.tensor.matmul(ps_i[:, hsl], exr[:, msl], P4h,
                                     start=False, stop=False)
                    nc.tensor.matmul(ps_r[:, hsl], exnegr[:, msl], P2h,
                                     start=False, stop=False)
                    nc.tensor.matmul(ps_r[:, hsl], exnegi[:, msl], P3h,
                                     start=False, stop=False)
                    nc.tensor.matmul(ps_r[:, hsl], exnegi[:, msl], P4h,
                                     start=False, stop=last)
                    nc.tensor.matmul(ps_i[:, hsl], exi[:, msl], P1h,
                                     start=False, stop=False)
                    nc.tensor.matmul(ps_i[:, hsl], exnegi[:, msl], P2h,
                                     start=False, stop=last)

            # ---- warp coords + hat matrices (overlap NUFFT on PE) ----
            pxf = cpool.tile([128, 128], F32, tag="pxf")
            nc.vector.tensor_scalar(pxf[:], fx_sb, xcol, 0.0, Alu.add, Alu.max)
            nc.vector.tensor_scalar_min(pxf[:], pxf[:], CLIP_HI)
            pyf = cpool.tile([128, 128], F32, tag="pyf")
            nc.vector.tensor_add(pyf[:], fy_sb, ramp)
            nc.vector.tensor_scalar(pyf[:], pyf[:], 0.0, CLIP_HI, Alu.max, Alu.min)
            pyf_neg = cpool.tile([128, 128], F32, tag="pyfn")
            nc.gpsimd.tensor_scalar_mul(pyf_neg[:], pyf[:], -1.0)

            # hx = min(|pxf - px|, 1) - 1 (NEGATED hat, DVE+Pool)
            # hy = relu(1 - |pyf - py|)  (positive hat, ACT)
            # sign balance: R is scaled by -img, so (-hx)^T @ (hy * -img) = +.
            hx_all = bpool.tile([128, NQ * 128], BF16, tag="hx")
            hy_all = bpool.tile([128, NQ * 128], BF16, tag="hy")
            for k in range(NQ):
                sl = slice(k * 128, (k + 1) * 128)
                vx = wpool.tile([128, 128], F32, tag="vx")
                nc.vector.tensor_scalar_sub(vx[:], ramp, pxf[:, k:k + 1])
                nc.vector.scalar_tensor_tensor(   # |vx| in place
                    vx[:], vx[:], -1.0, vx[:], Alu.mult, Alu.max)
                nc.gpsimd.tensor_scalar(
                    hx_all[:, sl], vx[:], 1.0, 1.0, Alu.min, Alu.subtract)
                h1y = wpool.tile([128, 128], F32, tag="h1y")
                nc.scalar.activation(
                    h1y[:], ramp, Act.Abs, bias=pyf_neg[:, k:k + 1], scale=1.0)
                nc.scalar.activation(
                    hy_all[:, sl], h1y[:], Act.Relu, bias=1.0, scale=-1.0)

            # ---- stage D: SENSE coil combine (wide mult + coil-tree add) ----
            def coil_combine(ps_a, cs_a, ps_b, cs_b, op_b, out_t):
                """out = sum_c cs_a*ps_a (+/-) cs_b*ps_b, tree-reduced over 8 coils."""
                w = cpool.tile([128, Nc * 128], F32, tag="sensew")
                nc.vector.tensor_tensor(w[:], ps_a[:], cs_a, Alu.mult)
                w2 = pspool.tile([128, Nc * 128], F32, tag="sensew2")
                nc.vector.tensor_tensor(w2[:], ps_b[:], cs_b, Alu.mult)
                nc.vector.tensor_tensor(w[:], w[:], w2[:], op_b)
                nc.vector.tensor_tensor(w[:, 0:512], w[:, 0:512], w[:, 512:1024], Alu.add)
                nc.vector.tensor_tensor(w[:, 0:256], w[:, 0:256], w[:, 256:512], Alu.add)
                nc.vector.tensor_tensor(out_t[:], w[:, 0:128], w[:, 128:256], Alu.add)

            img_r = cpool.tile([128, 128], F32, tag="imgr")
            img_i = cpool.tile([128, 128], F32, tag="imgi")
            coil_combine(ps_r, csr_sb, ps_i, csi_sb, Alu.add, img_r)
            coil_combine(ps_i, csr_sb, ps_r, csi_sb, Alu.subtract, img_i)

            # ---- stage E: warp-adjoint matmuls ----
            imgr_n = cpool.tile([128, 128], F32, tag="imgrn")
            nc.gpsimd.tensor_scalar_mul(imgr_n[:], img_r[:], -1.0)
            imgi_n = cpool.tile([128, 128], F32, tag="imgin")
            nc.gpsimd.tensor_scalar_mul(imgi_n[:], img_i[:], -1.0)
            ps_out = pspool.tile([128, 256], F32, tag="ps_out")
            for k in range(NQ):
                sl = slice(k * 128, (k + 1) * 128)
                R = wpool.tile([128, 256], BF16, tag="R")
                nc.vector.tensor_scalar_mul(R[:, 0:128], hy_all[:, sl], imgr_n[:, k:k + 1])
                nc.vector.tensor_scalar_mul(R[:, 128:256], hy_all[:, sl], imgi_n[:, k:k + 1])
                nc.tensor.matmul(ps_out[:], hx_all[:, sl], R[:],
                                 start=(k == 0), stop=(k == NQ - 1))

            out_sb = cpool.tile([128, 256], F32, tag="out_sb")
            nc.scalar.copy(out_sb[:], ps_out[:])
            nc.sync.dma_start(out[:], out_sb[:])

    _split_multiwaits(nc)
    return nc


def _prep_inputs(kspace, traj, csm, dcf, flow):
    """Shard full inputs into 8 per-core input maps (host-side, mechanical)."""
    kspace = np.asarray(kspace)
    traj = np.asarray(traj, dtype=np.float32)
    csm = np.asarray(csm)
    dcf = np.asarray(dcf, dtype=np.float32)
    flow = np.asarray(flow, dtype=np.float32)

    csr = np.ascontiguousarray(
        csm[0].real.astype(np.float32).transpose(1, 0, 2).reshape(128, Nc * 128))
    csi = np.ascontiguousarray(
        csm[0].imag.astype(np.float32).transpose(1, 0, 2).reshape(128, Nc * 128))

    f32 = np.float32
    ramp = np.broadcast_to(np.arange(128, dtype=f32)[None, :], (128, 128))
    rampc = ramp - f32(64.0)
    xcol = np.arange(128, dtype=f32)[:, None]
    bnpi = np.full((128, 1), -np.pi, f32)
    bppi = np.full((128, 1), np.pi, f32)

    in_maps = []
    for core in range(8):
        t, h = core // 2, core % 2
        msl = slice(h * MH, (h + 1) * MH)
        kx = traj[0, msl, 0, t].reshape(NCH, 128).T
        ky = traj[0, msl, 1, t].reshape(NCH, 128).T
        d = kspace[0, :, msl] * dcf[0, msl, t][None, :]
        dr = d.real.astype(f32).reshape(Nc, NCH, 128).transpose(2, 0, 1).reshape(128, Nc * NCH)
        di = d.imag.astype(f32).reshape(Nc, NCH, 128).transpose(2, 0, 1).reshape(128, Nc * NCH)
        inp = np.concatenate(
            [kx, ky, dr, di, csr, csi,
             flow[0, :, :, 0, t], flow[0, :, :, 1, t],
             ramp, rampc, xcol, bnpi, bppi], axis=1, dtype=f32)
        assert inp.shape == (128, NCOLS)
        in_maps.append({"inp": np.ascontiguousarray(inp)})
    return in_maps


def _run(inputs, trace=False):
    global _PROGRAM
    if _PROGRAM is None:
        _PROGRAM = _build_program()
    in_maps = _prep_inputs(**inputs)
    res = run_bass_kernel_spmd(_PROGRAM, in_maps, list(range(8)), trace=trace)
    acc = np.zeros((128, 256), np.float64)
    for r in res.results:
        acc += r["out"].astype(np.float64)
    acc = acc.astype(np.float32)
    full = np.stack([acc[:, 0:128], acc[:, 128:256]], axis=-1)[None]
    return full, res


def kernel(kspace, traj, csm, dcf, flow):
    full, _ = _run(dict(kspace=kspace, traj=traj, csm=csm, dcf=dcf, flow=flow))
    return full


# revision 47
# speedup vs baseline: 1.0031x; 1.0010x over previous
"""Trainium2 Bass kernel for motion-resolved NUFFT-adjoint (gpuNUFFT adj + warp).

Sharding: 8 cores = 4 motion states x 2 m-halves (each core: all 8 coils,
M/2=4096 k-space samples of one frame). Each core computes its partial
SENSE image and applies the frame's warp-adjoint (bilinear scatter-add,
expressed as two hat-matrix matmuls on the TensorEngine). Host sums the 8
partial outputs — no collectives needed since everything is linear in the
k-space data.

Device pipeline per core:
  A. phases psi = kx*(x-64) + 64 per m-chunk (DVE); range reduction
     t = psi - rint(psi) via the 2^23 magic-add (Pool, split into two
     single-op instructions: the chained-op ALU keeps an unrounded
     intermediate, and only positive sums round IEEE-correctly);
     sin(2*pi*t) on ACT Sin (valid input range [-pi, pi]), bf16 out
  B. fold: raw products P1..P4 = {eyr,eyi} x {dr,di}_c[m] only, via
     DVE tensor_scalar_mul / ACT Copy-with-scale-AP (bf16) - no
     elementwise complex combine at all
  C. NUFFT: the complex combine runs as 8 fp32 PSUM-accumulation
     chains on the PE with weight set {exr, exi, -exr, -exi} (the
     negated copies are free extra ACT sins with scale=-2pi):
       im_r += exr^T P1 - exr^T P2 - exi^T P3 - exi^T P4
       im_i += exr^T P3 + exr^T P4 + exi^T P1 - exi^T P2
  D. SENSE: img = sum_c conj(csm_c)*im_c, wide mults + coil-tree (DVE)
  E. warp adjoint: neg-hat hx = min(|pxf-px|,1)-1 (DVE+Pool) and
     pos-hat hy = relu(1-|pyf-py|) (ACT); R = hy * (-img_col);
     out += hx^T @ R accumulates over 128 q-chunks (PE), so the
     bilinear scatter-add runs entirely on the TensorEngine.

Engine-level notes baked in: every TPB instruction gets exactly one
sync-wait slot in this walrus (extra waits hoisted onto NoOps by
_split_multiwaits); Pool/ACT reject AP-scalar tensor_scalar, so all
per-partition-scalar elementwise work lives on DVE with immediate-only
passes on Pool and affine/bias-AP passes on ACT; hat emission is placed
after the fold so the Tile scheduler does not starve the NUFFT chain.
"""
import sys
import numpy as np

sys.path.insert(0, "/opt/trn_rl_repo")

import concourse.bass as bass
import concourse.mybir as mybir
import concourse.tile as tile
from concourse.bass_utils import run_bass_kernel_spmd

F32 = mybir.dt.float32
BF16 = mybir.dt.bfloat16
Alu = mybir.AluOpType
Act = mybir.ActivationFunctionType

Nc, M, Nt, Nx, Ny = 8, 8192, 4, 128, 128
MH = M // 2          # m-samples per core
NCH = MH // 128      # 32 m-chunks per core
NQ = Nx * Ny // 128  # 128 q-chunks (warp contraction)
TWO_PI = float(2.0 * np.pi)
PI = float(np.pi)
CLIP_HI = float(np.float32(Nx - 1.001))

SEGS = [("kx", NCH), ("ky", NCH), ("dr", Nc * NCH), ("di", Nc * NCH),
        ("csr", Nc * 128), ("csi", Nc * 128),
        ("fx", 128), ("fy", 128), ("ramp", 128), ("rampc", 128),
        ("xcol", 1), ("bnpi", 1), ("bppi", 1)]
OFFS = {}
_o = 0
for _n, _w in SEGS:
    OFFS[_n] = _o
    _o += _w
NCOLS = _o

_PROGRAM = None


def _split_multiwaits(nc):
    """Walrus codegen fits one sync-wait per TPB instruction; hoist extras
    into single-wait NoOps on the same engine, placed just before."""
    for bb in nc.main_func.blocks:
        new = []
        for ins in bb.instructions:
            si = getattr(ins, "sync_info", None)
            if si is not None and si.on_wait and len(si.on_wait) > 1:
                waits = list(si.on_wait)
                for w in waits[:-1]:
                    nop = mybir.InstNoOp(name=f"WSPLIT-{nc.next_id()}", ins=[], outs=[])
                    nop.engine = ins.engine
                    nop.sync_info = mybir.SyncInfo(on_wait=[w], on_update=[])
                    new.append(nop)
                ins.sync_info = mybir.SyncInfo(
                    on_wait=[waits[-1]], on_update=list(si.on_update))
            new.append(ins)
        bb.instructions[:] = new


def _build_program():
    nc = bass.Bass()

    inp = nc.declare_dram_parameter("inp", [128, NCOLS], F32, isOutput=False)
    out = nc.declare_dram_parameter("out", [128, 256], F32, isOutput=True)

    with tile.TileContext(nc) as tc:
        with (
            tc.tile_pool(name="const", bufs=1) as cpool,
            tc.tile_pool(name="big", bufs=1) as bpool,
            tc.tile_pool(name="ph", bufs=1) as phpool,
            tc.tile_pool(name="fold", bufs=3) as fpool,
            tc.tile_pool(name="warp", bufs=12) as wpool,
            tc.tile_pool(name="psum", bufs=1, space="PSUM") as pspool,
        ):
            # ---- single input DMA; all operands are slices of inp_sb ----
            inp_sb = cpool.tile([128, NCOLS], F32, tag="inp")
            nc.sync.dma_start(inp_sb[:], inp[:])
            # downstream ops otherwise each wait on several DMA-queue sems and
            # overflow the per-instruction sync-wait budget in walrus codegen
            tc.strict_bb_all_engine_barrier()

            def seg(name, n):
                o = OFFS[name]
                return inp_sb[:, o:o + n]

            kx_sb = seg("kx", NCH)
            ky_sb = seg("ky", NCH)
            dr_sb = seg("dr", Nc * NCH)
            di_sb = seg("di", Nc * NCH)
            csr_sb = seg("csr", Nc * 128)
            csi_sb = seg("csi", Nc * 128)
            fx_sb = seg("fx", 128)
            fy_sb = seg("fy", 128)
            ramp = seg("ramp", 128)     # row j -> j
            rampc = seg("rampc", 128)   # row j -> j-64
            xcol = seg("xcol", 1)       # partition p -> p

            # ---- stage A: phase tables + sincos ----
            MAGIC = float(2.0 ** 23)

            def build_exp(ktraj_sb, want_neg, inter):
                """bf16 sin/cos tables [128, MH] (m=128k+p, col k*128+x).
                psi = kx*(x-64) + 64 kept positive: the Pool/DVE adders only
                match IEEE round-to-nearest for positive magic-number sums."""
                ph = phpool.tile([128, MH], F32, tag="ph")
                for k in range(NCH):
                    nc.vector.tensor_scalar(
                        ph[:, k * 128:(k + 1) * 128], rampc,
                        ktraj_sb[:, k:k + 1], 64.0, Alu.mult, Alu.add)
                if inter is None:
                    ei = bpool.tile([128, MH], BF16, tag=f"ei{want_neg}")
                    er = bpool.tile([128, MH], BF16, tag=f"er{want_neg}")
                else:
                    # strided views into the interleaved tile: er -> even
                    # 128-col blocks, ei -> odd blocks (multi-dim APs)
                    v = inter[:].rearrange("p (k two x) -> p k two x", two=2, x=128)
                    er = v[:, :, 0, :]
                    ei = v[:, :, 1, :]
                if want_neg:
                    nei = bpool.tile([128, MH], BF16, tag="nei")
                    ner = bpool.tile([128, MH], BF16, tag="ner")
                else:
                    nei = None
                    ner = None
                QW = MH // 4
                for q in range(4):  # quarters pipeline Pool->DVE->ACT stages
                    qs = slice(q * QW, (q + 1) * QW)
                    phc = phpool.tile([128, QW], F32, tag="phcq")
                    nc.gpsimd.tensor_scalar_add(phc[:], ph[:, qs], 0.25)
                    t_s = phpool.tile([128, QW], F32, tag="tsq")
                    t_c = phpool.tile([128, QW], F32, tag="tcq")
                    for psi, t in ((ph[:, qs], t_s[:]), (phc[:], t_c[:])):
                        # k = rint(psi) via the 2^23 trick; two separate
                        # instructions so the write rounds to fp32
                        kk = phpool.tile([128, QW], F32, tag="kk")
                        nc.gpsimd.tensor_scalar_add(kk[:], psi, MAGIC)
                        nc.gpsimd.tensor_scalar_sub(kk[:], kk[:], MAGIC)
                        nc.gpsimd.tensor_sub(t, psi, kk[:])
                    if inter is None:
                        ei_q, er_q = ei[:, qs], er[:, qs]
                    else:
                        ei_q, er_q = ei[:, q * 8:(q + 1) * 8, :], er[:, q * 8:(q + 1) * 8, :]
                    nc.scalar.activation(ei_q, t_s[:], Act.Sin, bias=0.0, scale=TWO_PI)
                    nc.scalar.activation(er_q, t_c[:], Act.Sin, bias=0.0, scale=TWO_PI)
                    if want_neg:
                        nc.scalar.activation(nei[:, qs], t_s[:], Act.Sin, bias=0.0, scale=-TWO_PI)
                        nc.scalar.activation(ner[:, qs], t_c[:], Act.Sin, bias=0.0, scale=-TWO_PI)
                return er, ei, nei, ner

            # ey first: the fold (DVE-heavy) only needs ey; ex (matmul lhsT)
            # builds concurrently with the early fold chunks.
            # ey lands interleaved per chunk ([eyr_k | eyi_k] 256-col blocks)
            # so one FD=256 tensor_scalar yields two fold products at once.
            ey2 = bpool.tile([128, NCH * 256], BF16, tag="ey2")
            build_exp(ky_sb, False, ey2)
            exr, exi, exnegi, exnegr = build_exp(kx_sb, True, None)

            # ---- stages B+C: fold + NUFFT matmuls ----
            # raw products P1..P4; the complex combines run as 8 PSUM
            # accumulation chains on the PE (weights exr/exi/-exr/-exi)
            # im_r = exr*P1 - exr*P2 - exi*P3 - exi*P4
            # im_i = exr*P3 + exr*P4 + exi*P1 - exi*P2
            ps_r = pspool.tile([128, Nc * 128], F32, tag="ps_r")
            ps_i = pspool.tile([128, Nc * 128], F32, tag="ps_i")
            for k in range(NCH):
                bsl = slice(k * 256, (k + 1) * 256)
                P13 = fpool.tile([128, Nc * 256], BF16, tag="P13")
                P42 = fpool.tile([128, Nc * 256], BF16, tag="P42")
                for c in range(Nc):
                    csl = slice(c * 256, (c + 1) * 256)
                    dcol = dr_sb[:, c * NCH + k:c * NCH + k + 1]
                    icol = di_sb[:, c * NCH + k:c * NCH + k + 1]
                    if c < 2 and not (c == 0 and k % 3 == 0):
                        nc.scalar.activation(P13[:, csl], ey2[:, bsl], Act.Copy, scale=dcol)
                        nc.scalar.activation(P42[:, csl], ey2[:, bsl], Act.Copy, scale=icol)
                    else:
                        nc.vector.tensor_scalar_mul(P13[:, csl], ey2[:, bsl], dcol)
                        nc.vector.tensor_scalar_mul(P42[:, csl], ey2[:, bsl], icol)

                def half(buf, off, h):
                    # coils 4h..4h+3, inner 128 cols at `off` within each
                    # coil's 256-block -> [128, 4, 128] strided rhs (free 512)
                    v = buf[:].rearrange("p (c two x) -> p c two x", two=2, x=128)
                    return v[:, 4 * h:4 * h + 4, off, :]

                first, last = (k == 0), (k == NCH - 1)
                for h in range(2):  # N=512 column halves (4 coils each)
                    P1h, P3h = half(P13, 0, h), half(P13, 1, h)
                    P4h, P2h = half(P42, 0, h), half(P42, 1, h)
                    hsl = slice(h * 512, (h + 1) * 512)
                    msl = slice(k * 128, (k + 1) * 128)
                    nc.tensor.matmul(ps_r[:, hsl], exr[:, msl], P1h,
                                     start=first, stop=False)
                    nc.tensor.matmul(ps_i[:, hsl], exr[:, msl], P3h,
                                     start=first, stop=False)
                    nc.tensor.matmul(ps_i[:, hsl], exr[:, msl], P4h,
                                     start=False, stop=False)
                    nc.tensor.matmul(ps_r[:, hsl], exnegr[:, msl], P2h,
                                     start=False, stop=False)
                    nc.tensor.matmul(ps_r[:, hsl], exnegi[:, msl], P3h,
                                     start=False, stop=False)
                    nc.tensor.matmul(ps_r[:, hsl], exnegi[:, msl], P4h,
                                     start=False, stop=last)
                    nc.tensor.matmul(ps_i[:, hsl], exi[:, msl], P1h,
                                     start=False, stop=False)
                    nc.tensor.matmul(ps_i[:, hsl], exnegi[:, msl], P2h,
                                     start=False, stop=last)

            # ---- warp coords + hat matrices (overlap NUFFT on PE) ----
            pxf = cpool.tile([128, 128], F32, tag="pxf")
            nc.vector.tensor_scalar(pxf[:], fx_sb, xcol, 0.0, Alu.add, Alu.max)
            nc.vector.tensor_scalar_min(pxf[:], pxf[:], CLIP_HI)
            pyf = cpool.tile([128, 128], F32, tag="pyf")
            nc.vector.tensor_add(pyf[:], fy_sb, ramp)
            nc.vector.tensor_scalar(pyf[:], pyf[:], 0.0, CLIP_HI, Alu.max, Alu.min)
            pyf_neg = cpool.tile([128, 128], F32, tag="pyfn")
            nc.gpsimd.tensor_scalar_mul(pyf_neg[:], pyf[:], -1.0)

            # hx = min(|pxf - px|, 1) - 1 (NEGATED hat, DVE+Pool)
            # hy = relu(1 - |pyf - py|)  (positive hat, ACT)
            # sign balance: R is scaled by -img, so (-hx)^T @ (hy * -img) = +.
            hx_all = bpool.tile([128, NQ * 128], BF16, tag="hx")
            hy_all = bpool.tile([128, NQ * 128], BF16, tag="hy")
            for k in range(NQ):
                sl = slice(k * 128, (k + 1) * 128)
                vx = wpool.tile([128, 128], F32, tag="vx")
                nc.vector.tensor_scalar_sub(vx[:], ramp, pxf[:, k:k + 1])
                nc.vector.scalar_tensor_tensor(   # |vx| in place
                    vx[:], vx[:], -1.0, vx[:], Alu.mult, Alu.max)
                nc.gpsimd.tensor_scalar(
                    hx_all[:, sl], vx[:], 1.0, 1.0, Alu.min, Alu.subtract)
                h1y = wpool.tile([128, 128], F32, tag="h1y")
                nc.scalar.activation(
                    h1y[:], ramp, Act.Abs, bias=pyf_neg[:, k:k + 1], scale=1.0)
                nc.scalar.activation(
                    hy_all[:, sl], h1y[:], Act.Relu, bias=1.0, scale=-1.0)

            # ---- stage D: SENSE coil combine (wide mult + coil-tree add) ----
            def coil_combine(ps_a, cs_a, ps_b, cs_b, op_b, out_t):
                """out = sum_c cs_a*ps_a (+/-) cs_b*ps_b, tree-reduced over 8 coils."""
                w = cpool.tile([128, Nc * 128], F32, tag="sensew")
                nc.vector.tensor_tensor(w[:], ps_a[:], cs_a, Alu.mult)
                w2 = pspool.tile([128, Nc * 128], F32, tag="sensew2")
                nc.vector.tensor_tensor(w2[:], ps_b[:], cs_b, Alu.mult)
                nc.vector.tensor_tensor(w[:], w[:], w2[:], op_b)
                nc.vector.tensor_tensor(w[:, 0:512], w[:, 0:512], w[:, 512:1024], Alu.add)
                nc.vector.tensor_tensor(w[:, 0:256], w[:, 0:256], w[:, 256:512], Alu.add)
                nc.vector.tensor_tensor(out_t[:], w[:, 0:128], w[:, 128:256], Alu.add)

            img_r = cpool.tile([128, 128], F32, tag="imgr")
            img_i = cpool.tile([128, 128], F32, tag="imgi")
            coil_combine(ps_r, csr_sb, ps_i, csi_sb, Alu.add, img_r)
            coil_combine(ps_i, csr_sb, ps_r, csi_sb, Alu.subtract, img_i)

            # ---- stage E: warp-adjoint matmuls ----
            imgr_n = cpool.tile([128, 128], F32, tag="imgrn")
            nc.gpsimd.tensor_scalar_mul(imgr_n[:], img_r[:], -1.0)
            imgi_n = cpool.tile([128, 128], F32, tag="imgin")
            nc.gpsimd.tensor_scalar_mul(imgi_n[:], img_i[:], -1.0)
            ps_out = pspool.tile([128, 256], F32, tag="ps_out")
            for k in range(NQ):
                sl = slice(k * 128, (k + 1) * 128)
                R = wpool.tile([128, 256], BF16, tag="R")
                nc.vector.tensor_scalar_mul(R[:, 0:128], hy_all[:, sl], imgr_n[:, k:k + 1])
                nc.vector.tensor_scalar_mul(R[:, 128:256], hy_all[:, sl], imgi_n[:, k:k + 1])
                nc.tensor.matmul(ps_out[:], hx_all[:, sl], R[:],
                                 start=(k == 0), stop=(k == NQ - 1))

            out_sb = cpool.tile([128, 256], F32, tag="out_sb")
            nc.scalar.copy(out_sb[:], ps_out[:])
            nc.sync.dma_start(out[:], out_sb[:])

    _split_multiwaits(nc)
    return nc


def _prep_inputs(kspace, traj, csm, dcf, flow):
    """Shard full inputs into 8 per-core input maps (host-side, mechanical)."""
    kspace = np.asarray(kspace)
    traj = np.asarray(traj, dtype=np.float32)
    csm = np.asarray(csm)
    dcf = np.asarray(dcf, dtype=np.float32)
    flow = np.asarray(flow, dtype=np.float32)

    csr = np.ascontiguousarray(
        csm[0].real.astype(np.float32).transpose(1, 0, 2).reshape(128, Nc * 128))
    csi = np.ascontiguousarray(
        csm[0].imag.astype(np.float32).transpose(1, 0, 2).reshape(128, Nc * 128))

    f32 = np.float32
    ramp = np.broadcast_to(np.arange(128, dtype=f32)[None, :], (128, 128))
    rampc = ramp - f32(64.0)
    xcol = np.arange(128, dtype=f32)[:, None]
    bnpi = np.full((128, 1), -np.pi, f32)
    bppi = np.full((128, 1), np.pi, f32)

    in_maps = []
    for core in range(8):
        t, h = core // 2, core % 2
        msl = slice(h * MH, (h + 1) * MH)
        kx = traj[0, msl, 0, t].reshape(NCH, 128).T
        ky = traj[0, msl, 1, t].reshape(NCH, 128).T
        d = kspace[0, :, msl] * dcf[0, msl, t][None, :]
        dr = d.real.astype(f32).reshape(Nc, NCH, 128).transpose(2, 0, 1).reshape(128, Nc * NCH)
        di = d.imag.astype(f32).reshape(Nc, NCH, 128).transpose(2, 0, 1).reshape(128, Nc * NCH)
        inp = np.concatenate(
            [kx, ky, dr, di, csr, csi,
             flow[0, :, :, 0, t], flow[0, :, :, 1, t],
             ramp, rampc, xcol, bnpi, bppi], axis=1, dtype=f32)
        assert inp.shape == (128, NCOLS)
        in_maps.append({"inp": np.ascontiguousarray(inp)})
    return in_maps


def _run(inputs, trace=False):
    global _PROGRAM
    if _PROGRAM is None:
        _PROGRAM = _build_program()
    in_maps = _prep_inputs(**inputs)
    res = run_bass_kernel_spmd(_PROGRAM, in_maps, list(range(8)), trace=trace)
    acc = np.zeros((128, 256), np.float64)
    for r in res.results:
        acc += r["out"].astype(np.float64)
    acc = acc.astype(np.float32)
    full = np.stack([acc[:, 0:128], acc[:, 128:256]], axis=-1)[None]
    return full, res


def kernel(kspace, traj, csm, dcf, flow):
    full, _ = _run(dict(kspace=kspace, traj=traj, csm=csm, dcf=dcf, flow=flow))
    return full
